# revision 36
# baseline (speedup 1.0000x reference)
"""Multi-head attention (B=2,S=2048,D=1024,H=16) on 8 TRN2 NeuronCores.

Sharding: data-parallel over batch (2) x tensor-parallel over heads (4 groups
of 4 heads). Core c handles batch c//4, heads (c%4)*4 .. (c%4)*4+3.

Device layout strategy (no on-device transposes anywhere):
  - host passes x^T (d_model, S) per batch and w^T shards
  - qh/kh kept transposed per head: (d_k, S) "featT" layout
  - scores computed transposed: scoresT[kj, qi] = kh^T.T-free matmul
  - softmax denominator folded into the ctx matmul via an appended
    ones-column on V (row 64 of the ctx psum = row sums of exp(scores))
  - ctx computed transposed (d, qi), which feeds the out-projection
    directly as the stationary operand
  - attn output written transposed (kj, qi); host transposes on assembly
All matmul operands are float16 (1 cyc/row + FWL; psum accumulation f32).
"""

import numpy as np
from contextlib import ExitStack

import concourse.bass as bass
import concourse.tile as tile
from concourse import bacc, mybir
from concourse.bass_utils import run_bass_kernel_spmd

B, S, D, H = 2, 2048, 1024, 16
DK = 64
NCORES = 8
HL = 4          # heads per core
FL = HL * DK    # local features = 256
P = 128
KT = D // P     # 8 contraction tiles for projections
FT = FL // P    # 2 feature tiles
NKJ = S // P    # 16 key tiles
TT = S // P     # 16 token tiles
F32 = mybir.dt.float32
F32R = mybir.dt.float32r
F16 = mybir.dt.float16

_CACHE: dict = {}
TRACE: dict = {}  # test harness hook: {"kwargs": {...}} -> {"last": results}


def _build(causal: bool):
    NB = 512 if causal else 256   # qi block size
    NQ = S // NB

    nc = bacc.Bacc("TRN2", target_bir_lowering=False, debug=False,
                   enable_asserts=False, num_devices=NCORES)

    def din(name, shape, dt=F16):
        return nc.dram_tensor(name, shape, dt, kind="ExternalInput").ap()

    xq = din("xq", [D, S])
    xk = din("xk", [D, S])
    xv = din("xv", [D, S])
    wq = din("wq", [D, FL])
    wk = din("wk", [D, FL])
    wv = din("wv", [D, FL])
    wo = din("wo", [FL, D])
    bq = din("bq", [FT, P], F32)
    bk = din("bk", [FT, P], F32)
    bv = din("bv", [FL], F32)
    if causal:
        maskP = din("maskP", [P, NB + 384])
    else:
        maskT = din("maskT", [S, S])

    attnT = nc.dram_tensor("attnT", [HL, S, S], F16, kind="ExternalOutput").ap()
    outp = nc.dram_tensor("outp", [S, D], F32, kind="ExternalOutput").ap()
    # per-head reciprocal softmax denominators; host applies them to attnT
    recip_d = nc.dram_tensor("recip", [HL, S], F32, kind="ExternalOutput").ap()

    xq_t = xq.rearrange("(kt p) s -> p kt s", p=P)
    xk_t = xk.rearrange("(kt p) s -> p kt s", p=P)
    xv_t = xv.rearrange("(kt p) s -> p kt s", p=P)
    wq_t = wq.rearrange("(kt p) f -> p kt f", p=P)
    wk_t = wk.rearrange("(kt p) f -> p kt f", p=P)
    wv_t = wv.rearrange("(kt p) f -> p kt f", p=P)
    wo_t = wo.rearrange("(ft p) n -> p ft n", p=P)

    with tile.TileContext(nc) as tc, ExitStack() as ctx:
        consts = ctx.enter_context(tc.tile_pool(name="consts", bufs=1))
        xpool = ctx.enter_context(tc.tile_pool(name="xpool", bufs=4))
        apool = ctx.enter_context(tc.tile_pool(name="apool", bufs=2))
        spool = ctx.enter_context(tc.tile_pool(name="spool", bufs=2))
        psA = ctx.enter_context(tc.tile_pool(name="psA", bufs=2, space="PSUM"))
        psB = ctx.enter_context(tc.tile_pool(name="psB", bufs=1, space="PSUM"))

        # ---- constants ----
        wq_sb = consts.tile([P, KT, FL], F16)
        wk_sb = consts.tile([P, KT, FL], F16)
        wv_sb = consts.tile([P, KT, FL], F16)
        wo_sb = consts.tile([P, FT, D], F16)
        nc.sync.dma_start(out=wk_sb, in_=wk_t)
        nc.scalar.dma_start(out=wq_sb, in_=wq_t)
        nc.scalar.dma_start(out=wv_sb, in_=wv_t)
        nc.scalar.dma_start(out=wo_sb, in_=wo_t)
        bq_sb = consts.tile([P, FT], F32)
        bk_sb = consts.tile([P, FT], F32)
        nc.sync.dma_start(out=bq_sb, in_=bq.rearrange("ft p -> p ft"))
        nc.sync.dma_start(out=bk_sb, in_=bk.rearrange("ft p -> p ft"))
        bv_bc = consts.tile([P, FL], F32)
        nc.sync.dma_start(
            out=bv_bc,
            in_=bass.AP(tensor=bv.tensor, offset=bv.offset, ap=[[0, P]] + list(bv.ap)),
        )
        if causal:
            maskP_sb = consts.tile([P, NB + 384], F16)
            nc.scalar.dma_start(out=maskP_sb, in_=maskP)
        ones_f32 = consts.tile([P, P], F32)
        nc.vector.memset(ones_f32, 1.0)

        qhT_sb = consts.tile([P, FT, S], F16)
        khT_sb = consts.tile([P, FT, S], F16)
        ctxT_sb = consts.tile([P, FT, S], F16)
        vh_sb = consts.tile([P, TT, HL, DK + 1], F16)
        nc.vector.tensor_copy(
            vh_sb[:, :, :, DK:DK + 1],
            ones_f32[:, 0:TT * HL].rearrange("p (t h o) -> p t h o",
                                             t=TT, h=HL, o=1))

        # ---- K / Q projections (transposed: feat partitions, tokens free) ----
        def proj_T_block(x_t, w_sb, b_sb, dst_sb, pfx, qi, on_dve=False):
            x_sb = xpool.tile([P, KT, NB], F16, tag="xs", name=f"xs_{pfx}{qi}")
            nc.sync.dma_start(out=x_sb, in_=x_t[:, :, qi * NB:(qi + 1) * NB])
            for ft in range(FT):
                ps = psA.tile([P, NB], F32, tag="s", name=f"pp_{pfx}{qi}{ft}")
                for kt in range(KT):
                    nc.tensor.matmul(
                        ps, w_sb[:, kt, ft * P:(ft + 1) * P], x_sb[:, kt, :],
                        start=(kt == 0), stop=(kt == KT - 1))
                if on_dve:
                    nc.vector.tensor_scalar_add(
                        dst_sb[:, ft, qi * NB:(qi + 1) * NB], ps,
                        b_sb[:, ft:ft + 1])
                else:
                    nc.scalar.activation(
                        dst_sb[:, ft, qi * NB:(qi + 1) * NB], ps,
                        mybir.ActivationFunctionType.Identity,
                        bias=b_sb[:, ft:ft + 1], scale=1.0)

        def proj_T(x_t, w_sb, b_sb, dst_sb, pfx):
            for qi in range(NQ):
                proj_T_block(x_t, w_sb, b_sb, dst_sb, pfx, qi)

        proj_T(xk_t, wk_sb, bk_sb, khT_sb, "k")

        # ---- V projection (natural: token partitions, feature free) ----
        for t2 in range(TT // 2):
            xv_sb = xpool.tile([P, KT, 2, P], F16, tag="xv", name=f"xv_{t2}")
            nc.sync.dma_start(
                out=xv_sb,
                in_=xv_t[:, :, 2 * t2 * P:(2 * t2 + 2) * P]
                .rearrange("p kt (two q) -> p kt two q", q=P))
            for j in range(2):
                tt = 2 * t2 + j
                ps = psA.tile([P, FL], F32, tag="s", name=f"pv_{tt}")
                for kt in range(KT):
                    nc.tensor.matmul(ps, xv_sb[:, kt, j, :], wv_sb[:, kt, :],
                                     start=(kt == 0), stop=(kt == KT - 1))
                for h in range(HL):
                    nc.vector.tensor_add(
                        vh_sb[:, tt, h, 0:DK], ps[:, h * DK:(h + 1) * DK],
                        bv_bc[:, h * DK:(h + 1) * DK])

        proj_T_block(xq_t, wq_sb, bq_sb, qhT_sb, "q", 0)

        # ---- attention + output projection ----
        # Two heads' pipelines are interleaved per qi block so that one
        # head's softmax/normalize tail overlaps the other head's matmuls.
        def attn_head_block(qi, h, nkj):
            q0 = qi * NB
            p0 = (h % 2) * DK
            ft = h // 2
            ps_ctx = psB.tile([P, NB], F32, tag="ctx", bufs=2,
                              name=f"ctx_{qi}{h}")
            at_blk = apool.tile([P, NKJ, NB], F16, tag="attn", bufs=4,
                                name=f"atb_{qi}{h}")

            def trim(kj):
                # leading fully-masked columns of a diagonal tile
                d = kj * P - q0
                return d if (causal and d > 0) else 0

            def emit_scores(pr):
                kja, kjb = 2 * pr, 2 * pr + 1
                ps_s = psA.tile([P, 2, NB], F32, tag="s",
                                name=f"ss_{qi}{h}{pr}")
                for j, kj in enumerate((kja, kjb)):
                    t = trim(kj)
                    nc.tensor.matmul(
                        ps_s[:, j, t:],
                        khT_sb[p0:p0 + DK, ft, kj * P:(kj + 1) * P],
                        qhT_sb[p0:p0 + DK, ft, q0 + t:q0 + NB],
                        start=True, stop=True)
                return ps_s

            def emit_exp(pr, ps_s):
                kja, kjb = 2 * pr, 2 * pr + 1
                at = at_blk[:, kja:kjb + 1, :]
                nc.scalar.activation(at, ps_s,
                                     mybir.ActivationFunctionType.Exp,
                                     scale=0.125)
                for j, kj in enumerate((kja, kjb)):
                    k0 = kj * P
                    if causal and k0 > q0 - P:
                        t = trim(kj)
                        nc.vector.tensor_mul(
                            at[:, j, t:], at[:, j, t:],
                            maskP_sb[:, 384:384 + NB - t])
                if not causal:
                    mk = spool.tile([P, 2, NB], F16, tag="mk", bufs=4,
                                    name=f"mk_{qi}{h}{pr}")
                    nc.sync.dma_start(
                        out=mk,
                        in_=maskT[kja * P:(kjb + 1) * P, q0:q0 + NB]
                        .rearrange("(two p) n -> p two n", p=P))
                    nc.vector.tensor_mul(at, at, mk)
                return at

            def emit_ctx(pr, at):
                for j, kj in enumerate((2 * pr, 2 * pr + 1)):
                    t = trim(kj)
                    nc.tensor.matmul(
                        ps_ctx[0:DK + 1, t:], vh_sb[:, kj, h, :], at[:, j, t:],
                        start=(kj == 0), stop=(kj == nkj - 1))

            def emit_tail():
                nc.gpsimd.dma_start(
                    out=attnT[h, 0:nkj * P, q0:q0 + NB]
                    .rearrange("(kj p) n -> p kj n", p=P),
                    in_=at_blk[:, 0:nkj, :])
                sums = spool.tile([1, NB], F32, tag="sums", bufs=3,
                                  name=f"sm_{qi}{h}")
                nc.vector.tensor_copy(sums, ps_ctx[DK:DK + 1, :])
                recip = spool.tile([1, NB], F32, tag="recip", bufs=3,
                                   name=f"rc_{qi}{h}")
                nc.vector.reciprocal_approx_fast(recip, sums)
                nc.gpsimd.dma_start(out=recip_d[h, q0:q0 + NB], in_=recip)
                ps_bc = psB.tile([P, NB], F32, tag="out", bufs=2,
                                 name=f"bc_{qi}{h}")
                nc.tensor.matmul(ps_bc[0:DK, :], ones_f32[0:1, 0:DK], recip,
                                 start=True, stop=True)
                rb = spool.tile([DK, NB], F32, tag="rb", bufs=3,
                                name=f"rb_{qi}{h}")
                nc.vector.tensor_copy(rb, ps_bc[0:DK, :])
                nc.vector.tensor_mul(
                    ctxT_sb[p0:p0 + DK, ft, q0:q0 + NB], ps_ctx[0:DK, :], rb)

            return emit_scores, emit_exp, emit_ctx, emit_tail

        def make_outproj(qi):
            def emit():
                for t4 in range(NB // P):
                    t0 = qi * NB + t4 * P
                    for do in range(2):
                        ps_o = psB.tile([P, 512], F32, tag="out", bufs=2,
                                        name=f"po_{qi}{t4}{do}")
                        for ft in range(FT):
                            nc.tensor.matmul(
                                ps_o, ctxT_sb[:, ft, t0:t0 + P],
                                wo_sb[:, ft, do * 512:(do + 1) * 512],
                                start=(ft == 0), stop=(ft == FT - 1))
                        o_sb = spool.tile([P, 512], F32, tag="osb", bufs=3,
                                          name=f"ob_{qi}{t4}{do}")
                        nc.vector.tensor_copy(o_sb, ps_o)
                        nc.gpsimd.dma_start(
                            out=outp[t0:t0 + P, do * 512:(do + 1) * 512],
                            in_=o_sb)
            return emit

        pending_tail = None
        pending_outproj = None
        for qi in range(NQ):
            q0 = qi * NB
            nkj = min(NKJ, (q0 + NB) // P) if causal else NKJ
            npr = nkj // 2
            for h in range(HL):
                se, ee, ce, te = attn_head_block(qi, h, nkj)
                prev = None
                for pr in range(npr):
                    ps_s = se(pr)
                    if prev is not None:
                        ce(prev[0], prev[1])
                    if pr == 1 and pending_tail is not None:
                        # previous block's softmax tail: its DVE chain is done
                        # by now, so its bc-matmul slots in without stalling PE
                        pending_tail()
                        pending_tail = None
                    if pr == 2 and pending_outproj is not None:
                        pending_outproj()
                        pending_outproj = None
                    if pr == 0 and h == 2 and qi < NQ - 1:
                        proj_T_block(xq_t, wq_sb, bq_sb, qhT_sb, "q",
                                     qi + 1, on_dve=True)
                    at = ee(pr, ps_s)
                    prev = (pr, at)
                ce(prev[0], prev[1])
                if pending_tail is not None:
                    pending_tail()
                pending_tail = te
            pending_tail()
            pending_tail = None
            if pending_outproj is not None:
                pending_outproj()
            pending_outproj = make_outproj(qi)
        pending_outproj()

    nc.compile()
    return nc


def _get(causal: bool):
    if causal not in _CACHE:
        _CACHE[causal] = _build(causal)
    return _CACHE[causal]


def kernel(q, k, v, mask, w_q, b_q, w_k, b_k, w_v, b_v, w_o, b_o):
    q = np.ascontiguousarray(np.asarray(q), dtype=np.float32)
    k = np.ascontiguousarray(np.asarray(k), dtype=np.float32)
    v = np.ascontiguousarray(np.asarray(v), dtype=np.float32)
    mask = np.asarray(mask)
    w_q = np.asarray(w_q, dtype=np.float32)
    w_k = np.asarray(w_k, dtype=np.float32)
    w_v = np.asarray(w_v, dtype=np.float32)
    w_o = np.asarray(w_o, dtype=np.float32)
    b_q = np.asarray(b_q, dtype=np.float32)
    b_k = np.asarray(b_k, dtype=np.float32)
    b_v = np.asarray(b_v, dtype=np.float32)
    b_o = np.asarray(b_o, dtype=np.float32)

    m2 = np.asarray(mask).reshape(S, S)
    causal = bool(np.array_equal(m2 != 0, np.tril(np.ones((S, S), bool))))
    nc = _get(causal)

    xT = [np.ascontiguousarray(x[b].T.astype(np.float16))
          for x in (q, k, v) for b in range(B)]
    xqT, xkT, xvT = xT[0:2], xT[2:4], xT[4:6]
    w_q16 = w_q.T.astype(np.float16)
    w_k16 = w_k.T.astype(np.float16)
    w_v16 = w_v.T.astype(np.float16)
    w_o16 = w_o.T.astype(np.float16)

    if causal:
        NB = 512
        # maskP[kr, u] = 1.0 iff u >= kr + 384  (slice at 384-delta per tile)
        u = np.arange(NB + 384)[None, :]
        kr = np.arange(P)[:, None]
        maskP = (u >= kr + 384).astype(np.float16)
    else:
        maskTf = np.ascontiguousarray((m2 != 0).T.astype(np.float16))

    in_maps = []
    for c in range(NCORES):
        b, hg = divmod(c, 4)
        f0 = hg * FL
        im = {
            "xq": xqT[b], "xk": xkT[b], "xv": xvT[b],
            "wq": np.ascontiguousarray(w_q16[:, f0:f0 + FL]),
            "wk": np.ascontiguousarray(w_k16[:, f0:f0 + FL]),
            "wv": np.ascontiguousarray(w_v16[:, f0:f0 + FL]),
            "wo": np.ascontiguousarray(w_o16[f0:f0 + FL, :]),
            "bq": b_q[f0:f0 + FL].reshape(FT, P).copy(),
            "bk": b_k[f0:f0 + FL].reshape(FT, P).copy(),
            "bv": b_v[f0:f0 + FL].copy(),
        }
        if causal:
            im["maskP"] = maskP
        else:
            im["maskT"] = maskTf
        in_maps.append(im)

    kwargs = dict(TRACE.get("kwargs") or {})
    res = run_bass_kernel_spmd(nc, in_maps, core_ids=list(range(NCORES)), **kwargs)
    TRACE["last"] = res

    out = np.zeros((B, S, D), np.float32)
    attn = np.empty((B, H, S, S), np.float32)
    for c in range(NCORES):
        b, hg = divmod(c, 4)
        out[b] += res.results[c]["outp"]
        at = res.results[c]["attnT"]
        rc = res.results[c]["recip"]
        for j in range(HL):
            a = at[j].T.astype(np.float32) * rc[j][:, None]
            attn[b, hg * HL + j] = np.tril(a) if causal else a
    out += b_o[None, None, :]
    return out, attn


# revision 37
# speedup vs baseline: 1.0447x; 1.0447x over previous
"""Multi-head attention (B=2,S=2048,D=1024,H=16) on 8 TRN2 NeuronCores.

Sharding: data-parallel over batch (2) x tensor-parallel over heads (4 groups
of 4 heads). Core c handles batch c//4, heads (c%4)*4 .. (c%4)*4+3.

Device layout strategy (no on-device transposes anywhere):
  - host passes x^T (d_model, S) per batch and w^T shards
  - qh/kh kept transposed per head: (d_k, S) "featT" layout
  - scores computed transposed: scoresT[kj, qi] = kh^T.T-free matmul
  - softmax denominator folded into the ctx matmul via an appended
    ones-column on V (row 64 of the ctx psum = row sums of exp(scores))
  - ctx computed transposed (d, qi), which feeds the out-projection
    directly as the stationary operand
  - attn output written transposed (kj, qi); host transposes on assembly
All matmul operands are float16 (1 cyc/row + FWL; psum accumulation f32).
"""

import numpy as np
from contextlib import ExitStack

import concourse.bass as bass
import concourse.tile as tile
from concourse import bacc, mybir
from concourse.bass_utils import run_bass_kernel_spmd

B, S, D, H = 2, 2048, 1024, 16
DK = 64
NCORES = 8
HL = 4          # heads per core
FL = HL * DK    # local features = 256
P = 128
KT = D // P     # 8 contraction tiles for projections
FT = FL // P    # 2 feature tiles
NKJ = S // P    # 16 key tiles
TT = S // P     # 16 token tiles
F32 = mybir.dt.float32
F32R = mybir.dt.float32r
F16 = mybir.dt.float16

_CACHE: dict = {}
TRACE: dict = {}  # test harness hook: {"kwargs": {...}} -> {"last": results}


def _build(causal: bool):
    NB = 512 if causal else 256   # qi block size
    NQ = S // NB

    nc = bacc.Bacc("TRN2", target_bir_lowering=False, debug=False,
                   enable_asserts=False, num_devices=NCORES)

    def din(name, shape, dt=F16):
        return nc.dram_tensor(name, shape, dt, kind="ExternalInput").ap()

    xq = din("xq", [D, S])
    xk = din("xk", [D, S])
    xv = din("xv", [D, S])
    wq = din("wq", [D, FL])
    wk = din("wk", [D, FL])
    wv = din("wv", [D, FL])
    wo = din("wo", [FL, D])
    bq = din("bq", [FT, P], F32)
    bk = din("bk", [FT, P], F32)
    bv = din("bv", [FL], F32)
    if causal:
        maskP = din("maskP", [P, NB + 384])
    else:
        maskT = din("maskT", [S, S])

    attnT = nc.dram_tensor("attnT", [HL, S, S], F16, kind="ExternalOutput").ap()
    outp = nc.dram_tensor("outp", [S, D], F32, kind="ExternalOutput").ap()
    # per-head reciprocal softmax denominators; host applies them to attnT
    recip_d = nc.dram_tensor("recip", [HL, S], F32, kind="ExternalOutput").ap()

    xq_t = xq.rearrange("(kt p) s -> p kt s", p=P)
    xk_t = xk.rearrange("(kt p) s -> p kt s", p=P)
    xv_t = xv.rearrange("(kt p) s -> p kt s", p=P)
    wq_t = wq.rearrange("(kt p) f -> p kt f", p=P)
    wk_t = wk.rearrange("(kt p) f -> p kt f", p=P)
    wv_t = wv.rearrange("(kt p) f -> p kt f", p=P)
    wo_t = wo.rearrange("(ft p) n -> p ft n", p=P)

    with tile.TileContext(nc) as tc, ExitStack() as ctx:
        consts = ctx.enter_context(tc.tile_pool(name="consts", bufs=1))
        xpool = ctx.enter_context(tc.tile_pool(name="xpool", bufs=4))
        apool = ctx.enter_context(tc.tile_pool(name="apool", bufs=2))
        spool = ctx.enter_context(tc.tile_pool(name="spool", bufs=2))
        psA = ctx.enter_context(tc.tile_pool(name="psA", bufs=2, space="PSUM"))
        psB = ctx.enter_context(tc.tile_pool(name="psB", bufs=1, space="PSUM"))

        # ---- constants ----
        wq_sb = consts.tile([P, KT, FL], F16)
        wk_sb = consts.tile([P, KT, FL], F16)
        wv_sb = consts.tile([P, KT, FL], F16)
        wo_sb = consts.tile([P, FT, D], F16)
        nc.sync.dma_start(out=wk_sb, in_=wk_t)
        nc.scalar.dma_start(out=wq_sb, in_=wq_t)
        nc.scalar.dma_start(out=wv_sb, in_=wv_t)
        nc.scalar.dma_start(out=wo_sb, in_=wo_t)
        bq_sb = consts.tile([P, FT], F32)
        bk_sb = consts.tile([P, FT], F32)
        nc.sync.dma_start(out=bq_sb, in_=bq.rearrange("ft p -> p ft"))
        nc.sync.dma_start(out=bk_sb, in_=bk.rearrange("ft p -> p ft"))
        bv_bc = consts.tile([P, FL], F32)
        nc.sync.dma_start(
            out=bv_bc,
            in_=bass.AP(tensor=bv.tensor, offset=bv.offset, ap=[[0, P]] + list(bv.ap)),
        )
        if causal:
            maskP_sb = consts.tile([P, NB + 384], F16)
            nc.scalar.dma_start(out=maskP_sb, in_=maskP)
        ones_f32 = consts.tile([P, P], F32)
        nc.vector.memset(ones_f32, 1.0)

        qhT_sb = consts.tile([P, FT, S], F16)
        khT_sb = consts.tile([P, FT, S], F16)
        ctxT_sb = consts.tile([P, FT, S], F16)
        vh_sb = consts.tile([P, TT, HL, DK + 1], F16)
        nc.vector.tensor_copy(
            vh_sb[:, :, :, DK:DK + 1],
            ones_f32[:, 0:TT * HL].rearrange("p (t h o) -> p t h o",
                                             t=TT, h=HL, o=1))

        # ---- K / Q projections (transposed: feat partitions, tokens free) ----
        def proj_T_block(x_t, w_sb, b_sb, dst_sb, pfx, qi, on_dve=False):
            x_sb = xpool.tile([P, KT, NB], F16, tag="xs", name=f"xs_{pfx}{qi}")
            nc.sync.dma_start(out=x_sb, in_=x_t[:, :, qi * NB:(qi + 1) * NB])
            for ft in range(FT):
                ps = psA.tile([P, NB], F32, tag="s", name=f"pp_{pfx}{qi}{ft}")
                for kt in range(KT):
                    nc.tensor.matmul(
                        ps, w_sb[:, kt, ft * P:(ft + 1) * P], x_sb[:, kt, :],
                        start=(kt == 0), stop=(kt == KT - 1))
                if on_dve:
                    nc.vector.tensor_scalar_add(
                        dst_sb[:, ft, qi * NB:(qi + 1) * NB], ps,
                        b_sb[:, ft:ft + 1])
                else:
                    nc.scalar.activation(
                        dst_sb[:, ft, qi * NB:(qi + 1) * NB], ps,
                        mybir.ActivationFunctionType.Identity,
                        bias=b_sb[:, ft:ft + 1], scale=1.0)

        def proj_T(x_t, w_sb, b_sb, dst_sb, pfx):
            for qi in range(NQ):
                proj_T_block(x_t, w_sb, b_sb, dst_sb, pfx, qi)

        proj_T(xk_t, wk_sb, bk_sb, khT_sb, "k")

        # ---- V projection (natural: token partitions, feature free) ----
        for t2 in range(TT // 2):
            xv_sb = xpool.tile([P, KT, 2, P], F16, tag="xv", name=f"xv_{t2}")
            nc.sync.dma_start(
                out=xv_sb,
                in_=xv_t[:, :, 2 * t2 * P:(2 * t2 + 2) * P]
                .rearrange("p kt (two q) -> p kt two q", q=P))
            for j in range(2):
                tt = 2 * t2 + j
                ps = psA.tile([P, FL], F32, tag="s", name=f"pv_{tt}")
                for kt in range(KT):
                    nc.tensor.matmul(ps, xv_sb[:, kt, j, :], wv_sb[:, kt, :],
                                     start=(kt == 0), stop=(kt == KT - 1))
                for h in range(HL):
                    nc.vector.tensor_add(
                        vh_sb[:, tt, h, 0:DK], ps[:, h * DK:(h + 1) * DK],
                        bv_bc[:, h * DK:(h + 1) * DK])

        proj_T_block(xq_t, wq_sb, bq_sb, qhT_sb, "q", 0)

        # ---- attention + output projection ----
        # Two heads' pipelines are interleaved per qi block so that one
        # head's softmax/normalize tail overlaps the other head's matmuls.
        def attn_head_block(qi, h, nkj):
            q0 = qi * NB
            p0 = (h % 2) * DK
            ft = h // 2
            ps_ctx = psB.tile([P, NB], F32, tag="ctx", bufs=2,
                              name=f"ctx_{qi}{h}")
            at_blk = apool.tile([P, NKJ, NB], F16, tag="attn", bufs=4,
                                name=f"atb_{qi}{h}")

            def trim(kj):
                # leading fully-masked columns of a diagonal tile
                d = kj * P - q0
                return d if (causal and d > 0) else 0

            def emit_scores(pr):
                kja, kjb = 2 * pr, 2 * pr + 1
                ps_s = psA.tile([P, 2, NB], F32, tag="s",
                                name=f"ss_{qi}{h}{pr}")
                for j, kj in enumerate((kja, kjb)):
                    nc.tensor.matmul(
                        ps_s[:, j, :],
                        khT_sb[p0:p0 + DK, ft, kj * P:(kj + 1) * P],
                        qhT_sb[p0:p0 + DK, ft, q0:q0 + NB],
                        start=True, stop=True)
                return ps_s

            def emit_exp(pr, ps_s):
                kja, kjb = 2 * pr, 2 * pr + 1
                at = at_blk[:, kja:kjb + 1, :]
                nc.scalar.activation(at, ps_s,
                                     mybir.ActivationFunctionType.Exp,
                                     scale=0.125)
                for j, kj in enumerate((kja, kjb)):
                    k0 = kj * P
                    if causal and k0 > q0 - P:
                        d0 = 384 - (k0 - q0)
                        nc.vector.tensor_mul(at[:, j, :], at[:, j, :],
                                             maskP_sb[:, d0:d0 + NB])
                if not causal:
                    mk = spool.tile([P, 2, NB], F16, tag="mk", bufs=4,
                                    name=f"mk_{qi}{h}{pr}")
                    nc.sync.dma_start(
                        out=mk,
                        in_=maskT[kja * P:(kjb + 1) * P, q0:q0 + NB]
                        .rearrange("(two p) n -> p two n", p=P))
                    nc.vector.tensor_mul(at, at, mk)
                return at

            def emit_ctx(pr, at):
                for j, kj in enumerate((2 * pr, 2 * pr + 1)):
                    nc.tensor.matmul(
                        ps_ctx[0:DK + 1, :], vh_sb[:, kj, h, :], at[:, j, :],
                        start=(kj == 0), stop=(kj == nkj - 1))

            def emit_tail():
                nc.gpsimd.dma_start(
                    out=attnT[h, 0:nkj * P, q0:q0 + NB]
                    .rearrange("(kj p) n -> p kj n", p=P),
                    in_=at_blk[:, 0:nkj, :])
                sums = spool.tile([1, NB], F32, tag="sums", bufs=3,
                                  name=f"sm_{qi}{h}")
                nc.vector.tensor_copy(sums, ps_ctx[DK:DK + 1, :])
                recip = spool.tile([1, NB], F32, tag="recip", bufs=3,
                                   name=f"rc_{qi}{h}")
                nc.vector.reciprocal_approx_fast(recip, sums)
                nc.gpsimd.dma_start(out=recip_d[h, q0:q0 + NB], in_=recip)
                ps_bc = psB.tile([P, NB], F32, tag="out", bufs=2,
                                 name=f"bc_{qi}{h}")
                nc.tensor.matmul(ps_bc[0:DK, :], ones_f32[0:1, 0:DK], recip,
                                 start=True, stop=True)
                rb = spool.tile([DK, NB], F32, tag="rb", bufs=3,
                                name=f"rb_{qi}{h}")
                nc.vector.tensor_copy(rb, ps_bc[0:DK, :])
                nc.vector.tensor_mul(
                    ctxT_sb[p0:p0 + DK, ft, q0:q0 + NB], ps_ctx[0:DK, :], rb)

            return emit_scores, emit_exp, emit_ctx, emit_tail

        def make_outproj(qi):
            def emit():
                for t4 in range(NB // P):
                    t0 = qi * NB + t4 * P
                    for do in range(2):
                        ps_o = psB.tile([P, 512], F32, tag="out", bufs=2,
                                        name=f"po_{qi}{t4}{do}")
                        for ft in range(FT):
                            nc.tensor.matmul(
                                ps_o, ctxT_sb[:, ft, t0:t0 + P],
                                wo_sb[:, ft, do * 512:(do + 1) * 512],
                                start=(ft == 0), stop=(ft == FT - 1))
                        o_sb = spool.tile([P, 512], F32, tag="osb", bufs=3,
                                          name=f"ob_{qi}{t4}{do}")
                        nc.vector.tensor_copy(o_sb, ps_o)
                        nc.gpsimd.dma_start(
                            out=outp[t0:t0 + P, do * 512:(do + 1) * 512],
                            in_=o_sb)
            return emit

        pending_tail = None
        pending_outproj = None
        for qi in range(NQ):
            q0 = qi * NB
            nkj = min(NKJ, (q0 + NB) // P) if causal else NKJ
            npr = nkj // 2
            for h in range(HL):
                se, ee, ce, te = attn_head_block(qi, h, nkj)
                prev = None
                for pr in range(npr):
                    ps_s = se(pr)
                    if prev is not None:
                        ce(prev[0], prev[1])
                    if pr == 1 and pending_tail is not None:
                        # previous block's softmax tail: its DVE chain is done
                        # by now, so its bc-matmul slots in without stalling PE
                        pending_tail()
                        pending_tail = None
                    if pr == 2 and pending_outproj is not None:
                        pending_outproj()
                        pending_outproj = None
                    if pr == 0 and h == 2 and qi < NQ - 1:
                        proj_T_block(xq_t, wq_sb, bq_sb, qhT_sb, "q",
                                     qi + 1, on_dve=True)
                    at = ee(pr, ps_s)
                    prev = (pr, at)
                ce(prev[0], prev[1])
                if pending_tail is not None:
                    pending_tail()
                pending_tail = te
            pending_tail()
            pending_tail = None
            if pending_outproj is not None:
                pending_outproj()
            pending_outproj = make_outproj(qi)
        pending_outproj()

    nc.compile()
    return nc


def _get(causal: bool):
    if causal not in _CACHE:
        _CACHE[causal] = _build(causal)
    return _CACHE[causal]


def kernel(q, k, v, mask, w_q, b_q, w_k, b_k, w_v, b_v, w_o, b_o):
    q = np.ascontiguousarray(np.asarray(q), dtype=np.float32)
    k = np.ascontiguousarray(np.asarray(k), dtype=np.float32)
    v = np.ascontiguousarray(np.asarray(v), dtype=np.float32)
    mask = np.asarray(mask)
    w_q = np.asarray(w_q, dtype=np.float32)
    w_k = np.asarray(w_k, dtype=np.float32)
    w_v = np.asarray(w_v, dtype=np.float32)
    w_o = np.asarray(w_o, dtype=np.float32)
    b_q = np.asarray(b_q, dtype=np.float32)
    b_k = np.asarray(b_k, dtype=np.float32)
    b_v = np.asarray(b_v, dtype=np.float32)
    b_o = np.asarray(b_o, dtype=np.float32)

    m2 = np.asarray(mask).reshape(S, S)
    causal = bool(np.array_equal(m2 != 0, np.tril(np.ones((S, S), bool))))
    nc = _get(causal)

    xT = [np.ascontiguousarray(x[b].T.astype(np.float16))
          for x in (q, k, v) for b in range(B)]
    xqT, xkT, xvT = xT[0:2], xT[2:4], xT[4:6]
    w_q16 = w_q.T.astype(np.float16)
    w_k16 = w_k.T.astype(np.float16)
    w_v16 = w_v.T.astype(np.float16)
    w_o16 = w_o.T.astype(np.float16)

    if causal:
        NB = 512
        # maskP[kr, u] = 1.0 iff u >= kr + 384  (slice at 384-delta per tile)
        u = np.arange(NB + 384)[None, :]
        kr = np.arange(P)[:, None]
        maskP = (u >= kr + 384).astype(np.float16)
    else:
        maskTf = np.ascontiguousarray((m2 != 0).T.astype(np.float16))

    in_maps = []
    for c in range(NCORES):
        b, hg = divmod(c, 4)
        f0 = hg * FL
        im = {
            "xq": xqT[b], "xk": xkT[b], "xv": xvT[b],
            "wq": np.ascontiguousarray(w_q16[:, f0:f0 + FL]),
            "wk": np.ascontiguousarray(w_k16[:, f0:f0 + FL]),
            "wv": np.ascontiguousarray(w_v16[:, f0:f0 + FL]),
            "wo": np.ascontiguousarray(w_o16[f0:f0 + FL, :]),
            "bq": b_q[f0:f0 + FL].reshape(FT, P).copy(),
            "bk": b_k[f0:f0 + FL].reshape(FT, P).copy(),
            "bv": b_v[f0:f0 + FL].copy(),
        }
        if causal:
            im["maskP"] = maskP
        else:
            im["maskT"] = maskTf
        in_maps.append(im)

    kwargs = dict(TRACE.get("kwargs") or {})
    res = run_bass_kernel_spmd(nc, in_maps, core_ids=list(range(NCORES)), **kwargs)
    TRACE["last"] = res

    out = np.zeros((B, S, D), np.float32)
    attn = np.empty((B, H, S, S), np.float32)
    for c in range(NCORES):
        b, hg = divmod(c, 4)
        out[b] += res.results[c]["outp"]
        at = res.results[c]["attnT"]
        rc = res.results[c]["recip"]
        for j in range(HL):
            a = at[j].T.astype(np.float32) * rc[j][:, None]
            attn[b, hg * HL + j] = np.tril(a) if causal else a
    out += b_o[None, None, :]
    return out, attn
